# revision 27
# baseline (speedup 1.0000x reference)
"""Fused Trainium2 Bass kernel for a dense transformer block (v3, pipelined).

Reference computation (per batch b):
    h  = LN(x; g1, be1)                  # layernorm over C
    q,k,v = h @ Wq|Wk|Wv (per head)      # [T, HS] each, 6 heads
    wei = softmax(causal(q k^T / sqrt(HS)))
    o  = wei @ v (concat heads)          # [T, C]
    x  = x + o @ Wp + bp
    h2 = LN(x; g2, be2)
    out = x + relu(h2 @ W1 + b1) @ W2 + b2

Sharding: data-parallel over batch. B=64 -> 8 NeuronCores x 8 batches.
No collectives; each core runs an identical program on its own shard.

v3 design (N = 8*256 = 2048 tokens/core):
  - bf16 matmul operands throughout (PSUM accumulation stays fp32):
    fp32 LDWEIGHTS costs ~223ns/matmul and cannot use Fast Weight Load;
    bf16 LDW is ~53ns and hides behind the matmul stream.  bf16 also
    runs 1 cyc/row at ANY moving size (fp32r needs >=256), which makes
    the small A@V (N=66) matmuls 4x faster.
  - full phase pipelining: for each 512-token chunk j the program emits
    LN1 tiles -> FFN(j-1) -> QKV(j) -> attention(2j), attention(2j+1)
    (incl. Wp + LN2 per batch).  The tensor engine never idles long
    enough for the HAM clock-gate to re-throttle, and the old 45us
    PE-idle prologue overlaps with LN1/weight prep.
  - weight prep is emitted in pipeline order per engine so a fold that
    waits on a late DMA never head-of-line blocks LN1's engine queues.
  - one SBUF slab carries hT -> oT -> h2T per token range (WAR-chained).
  - x is DMA'd once and cached in SBUF (reused for the Wp residual).
  - big strided DMAs (one per weight matrix / 4-tile x chunk) instead of
    per-128-row transfers; out staged per 512-token chunk.
  - engine balance: LN applies + exp + half the evictions on ACT,
    stats/normalize/rest on DVE, causal masks + broadcast bias adds on
    GPSIMD; per-feature biases fused into PSUM evictions.
"""

import sys

if "/opt/trn_rl_repo" not in sys.path:
    sys.path.insert(0, "/opt/trn_rl_repo")

import numpy as np

import concourse.bacc as bacc
import concourse.bass as bass
import concourse.mybir as mybir
import concourse.tile as tile
from concourse.masks import make_identity

AF = mybir.ActivationFunctionType
ALU = mybir.AluOpType
F32 = mybir.dt.float32
F32R = mybir.dt.float32r
BF16 = mybir.dt.bfloat16

B, T, C, H = 64, 256, 384, 6
HS = C // H  # 64
VW = HS + 2  # v row padded: 64 v + 1 ones + 1 zero
EPS = 1e-3
NCORES = 8
BB = B // NCORES  # 8 batches/core
N = BB * T  # 2048 tokens/core
NT = N // 128  # 16 token tiles
NC3 = C // 128  # 3 chunks of C
F = 4 * C  # 1536
NF = F // 128  # 12 chunks of F
NJ = N // 512  # 4 token chunks (1 chunk = 2 batches)
ATT_SCALE = float(HS) ** -0.5

# per-tensor matmul-operand dtypes (precision knobs).
# walrus: if either matmul operand is f32/f32r, BOTH must match, so the
# slab (h/oT/h2T) and every weight it meets form one f32r class; bf16 is
# kept where it buys the most speed (fast LDW / small-N matmuls) within
# the ~2e-2 error budget: qk evictions, exp(scores), v, hid, W2.
DT_SLAB = F32R  # LN1 h / attention oT / LN2 h2 (transposed slab)
DT_QK = BF16    # qT / kT
DT_E = BF16     # exp(scores)
DT_V = BF16     # v
DT_W = F32R     # Wq/Wk/Wv/Wp
DT_W1 = F32R    # W1
DT_W2 = BF16    # W2
DT_HID = BF16   # FFN hidden


def build_nc():
    nc = bacc.Bacc()

    x_d = nc.declare_dram_parameter("x", [N, C], F32, isOutput=False)
    wq_d = nc.declare_dram_parameter("wq", [C, C], F32, isOutput=False)
    wk_d = nc.declare_dram_parameter("wk", [C, C], F32, isOutput=False)
    wv_d = nc.declare_dram_parameter("wv", [C, C], F32, isOutput=False)
    wp_d = nc.declare_dram_parameter("wp", [C, C], F32, isOutput=False)
    w1_d = nc.declare_dram_parameter("w1", [C, F], F32, isOutput=False)
    w2_d = nc.declare_dram_parameter("w2", [F, C], F32, isOutput=False)
    g1c_d = nc.declare_dram_parameter("g1c", [128, NC3], F32, isOutput=False)
    be1c_d = nc.declare_dram_parameter("be1c", [128, NC3], F32, isOutput=False)
    g2c_d = nc.declare_dram_parameter("g2c", [128, NC3], F32, isOutput=False)
    be2c_d = nc.declare_dram_parameter("be2c", [128, NC3], F32, isOutput=False)
    b1c_d = nc.declare_dram_parameter("b1c", [128, NF], F32, isOutput=False)
    bp_row_d = nc.declare_dram_parameter("bp_row", [1, C], F32, isOutput=False)
    b2_row_d = nc.declare_dram_parameter("b2_row", [1, C], F32, isOutput=False)
    out_d = nc.declare_dram_parameter("out", [N, C], F32, isOutput=True)

    # dram views for the big strided loads
    x_v = x_d.rearrange("(i p) c -> p i c", p=128)      # [128, 16, C]
    wq_v = wq_d.rearrange("(a p) c -> p a c", p=128)    # [128, 3, C]
    wk_v = wk_d.rearrange("(a p) c -> p a c", p=128)
    wv_v = wv_d.rearrange("(a p) c -> p a c", p=128)
    wp_v = wp_d.rearrange("(a p) c -> p a c", p=128)
    w1_v = w1_d.rearrange("(a p) f -> p a f", p=128)    # [128, 3, F]
    w2_v = w2_d.rearrange("(a p) c -> p a c", p=128)    # [128, 12, C]
    out_v = out_d.rearrange("(i p) c -> p i c", p=128)  # [128, 16, C]

    with tile.TileContext(nc) as tc:
        with (
            tc.tile_pool(name="const", bufs=1) as constp,
            tc.tile_pool(name="wts", bufs=1) as wts,
            tc.tile_pool(name="persist", bufs=1) as persist,
            tc.tile_pool(name="stage", bufs=2) as stagep,
            tc.tile_pool(name="stats", bufs=6) as stats,
            tc.tile_pool(name="hb", bufs=2) as hb,
            tc.tile_pool(name="expb", bufs=6) as expb,
            tc.tile_pool(name="ost", bufs=2) as ostp,
            tc.tile_pool(name="hid", bufs=2) as hidp,
            tc.tile_pool(name="outb", bufs=1) as outbp,
            tc.tile_pool(name="pmix", bufs=2, space="PSUM") as pbig,
            tc.tile_pool(name="pffn", bufs=2, space="PSUM") as pffn,
            tc.tile_pool(name="pw", bufs=2, space="PSUM") as pwp,
            tc.tile_pool(name="pov", bufs=2, space="PSUM") as povp,
        ):
            # ---------------- constants ----------------
            ident_f = constp.tile([128, 128], F32)
            make_identity(nc, ident_f)
            ident = constp.tile([128, 128], F32R)
            nc.vector.tensor_copy(ident, ident_f)
            eps_col = constp.tile([128, 1], F32)
            nc.vector.memset(eps_col, EPS)
            ones128 = constp.tile([128, 128], F32)
            nc.vector.memset(ones128, 1.0)
            zero128 = constp.tile([128, 128], F32)
            nc.vector.memset(zero128, 0.0)
            # causal keep-mask for diagonal blocks: keep where j - p >= 0
            mask01 = constp.tile([128, 128], F32)
            nc.gpsimd.affine_select(
                out=mask01,
                in_=ones128,
                pattern=[[1, 128]],
                compare_op=ALU.is_ge,
                fill=0.0,
                base=0,
                channel_multiplier=-1,
            )
            mask2x = constp.tile([128, 256], BF16)
            nc.vector.tensor_copy(mask2x[:, 0:128], mask01)
            nc.vector.tensor_copy(mask2x[:, 128:256], mask01)

            # ---------------- persistent tensors ----------------
            # slab carries hT -> oT -> h2T (per token range, WAR-chained)
            slab = persist.tile([128, NC3, N], DT_SLAB)
            qT = persist.tile([128, NC3, N], DT_QK)
            kT = persist.tile([128, NC3, N], DT_QK)
            v1 = persist.tile([128, NT, H, VW], DT_V)
            # xs carries x, then (in place) y = x + sa after the Wp stage
            xs = persist.tile([128, NT, C], F32)

            wq = wts.tile([128, NC3, C], DT_W)
            wk = wts.tile([128, NC3, C], DT_W)
            wv = wts.tile([128, NC3, C], DT_W)
            wp = wts.tile([128, NC3, C], DT_W)
            w1 = wts.tile([128, NC3, F], DT_W1)
            w2 = wts.tile([128, NF, C], DT_W2)

            g1c = wts.tile([128, NC3], F32)
            be1c_f = wts.tile([128, NC3], F32)
            g2c = wts.tile([128, NC3], F32)
            be2c_f = wts.tile([128, NC3], F32)
            b1c = wts.tile([128, NF], F32)
            bp_f = wts.tile([1, C], F32)
            b2_f = wts.tile([1, C], F32)

            # ---------------- DMA queue (sync engine, in need-order) ------
            # params first: the weight folds depend on them, and a blocked
            # weight-stage trigger would head-of-line block the sync queue.
            for sb, dd in (
                (g1c, g1c_d), (be1c_f, be1c_d), (g2c, g2c_d), (be2c_f, be2c_d),
                (b1c, b1c_d), (bp_f, bp_row_d), (b2_f, b2_row_d),
            ):
                nc.sync.dma_start(sb, dd[:, :])
            nc.sync.dma_start(xs[:, 0:4, :], x_v[:, 0:4, :])
            wq_raw = stagep.tile([128, NC3, C], F32, tag="wst", bufs=2)
            nc.sync.dma_start(wq_raw, wq_v)
            wk_raw = stagep.tile([128, NC3, C], F32, tag="wst", bufs=2)
            nc.sync.dma_start(wk_raw, wk_v)
            nc.sync.dma_start(xs[:, 4:8, :], x_v[:, 4:8, :])
            wv_raw = stagep.tile([128, NC3, C], F32, tag="wst", bufs=2)
            nc.sync.dma_start(wv_raw, wv_v)
            wp_raw = stagep.tile([128, NC3, C], F32, tag="wst", bufs=2)
            nc.sync.dma_start(wp_raw, wp_v)
            for jx in range(2, 4):
                nc.sync.dma_start(
                    xs[:, 4 * jx : 4 * jx + 4, :], x_v[:, 4 * jx : 4 * jx + 4, :]
                )
            w1_raws = []
            for a in range(NC3):
                w1_raw = stagep.tile([128, F], F32, tag="w1st", bufs=1)
                nc.sync.dma_start(w1_raw, w1_v[:, a, :])
                w1_raws.append(w1_raw)
            w2_raws = []
            for half in range(2):
                w2_raw = stagep.tile([128, 6, C], F32, tag="w2st", bufs=1)
                nc.sync.dma_start(w2_raw, w2_v[:, 6 * half : 6 * half + 6, :])
                w2_raws.append(w2_raw)

            # be pairs + bias holders (cheap, deps ready instantly)
            be1c = wts.tile([128, NC3, 2], F32R)
            be2c = wts.tile([128, NC3, 2], F32R)
            nc.vector.tensor_copy(
                be1c[:, :, 0:1].rearrange("p c o -> p (c o)"), be1c_f
            )
            nc.vector.tensor_copy(
                be1c[:, :, 1:2].rearrange("p c o -> p (c o)"), zero128[:, 0:NC3]
            )
            nc.vector.tensor_copy(
                be2c[:, :, 0:1].rearrange("p c o -> p (c o)"), be2c_f
            )
            nc.vector.tensor_copy(
                be2c[:, :, 1:2].rearrange("p c o -> p (c o)"), zero128[:, 0:NC3]
            )
            bqc = wts.tile([128, NC3], F32)
            bkc = wts.tile([128, NC3], F32)
            bv_row = wts.tile([1, C], F32)
            bvb = wts.tile([128, C], F32)
            bpb = wts.tile([128, C], F32)
            b2b = wts.tile([128, C], F32)
            b1tot = wts.tile([128, NF], F32)
            nc.gpsimd.partition_broadcast(bpb, bp_f)
            nc.gpsimd.partition_broadcast(b2b, b2_f)

            # v ones/zero columns
            nc.gpsimd.tensor_copy(
                v1[:, :, :, HS : HS + 1].rearrange("p i h o -> p (i h o)"),
                ones128[:, 0 : NT * H],
            )
            nc.gpsimd.tensor_copy(
                v1[:, :, :, HS + 1 : HS + 2].rearrange("p i h o -> p (i h o)"),
                zero128[:, 0 : NT * H],
            )

            # ---------------- deferred weight prep --------------------
            g1s = wts.tile([128, NC3], F32)

            def emit_prep_q():
                # folds on ACT (prologue DVE is busy with LN1 stats);
                # g1s pre-scales the attention scale into wq's gamma
                nc.vector.tensor_scalar(g1s, g1c, ATT_SCALE, None, ALU.mult)
                for a in range(NC3):
                    nc.scalar.activation(
                        wq[:, a, :], wq_raw[:, a, :], AF.Identity,
                        bias=0.0, scale=g1s[:, a : a + 1],
                    )

            def emit_prep_k():
                for a in range(NC3):
                    nc.scalar.activation(
                        wk[:, a, :], wk_raw[:, a, :], AF.Identity,
                        bias=0.0, scale=g1c[:, a : a + 1],
                    )

            def emit_prep_v():
                for a in range(NC3):
                    nc.scalar.activation(
                        wv[:, a, :], wv_raw[:, a, :], AF.Identity,
                        bias=0.0, scale=g1c[:, a : a + 1],
                    )
                # bias columns (per-feature) for qT/kT evictions: bq = Wq'^T be1
                for m in range(NC3):
                    pb = pffn.tile([128, 512], F32, tag="pf")
                    for c in range(NC3):
                        nc.tensor.matmul(
                            pb[:, 0:2], wq[:, c, m * 128 : (m + 1) * 128],
                            be1c[:, c, :],
                            start=(c == 0), stop=(c == NC3 - 1),
                        )
                    nc.vector.tensor_copy(bqc[:, m : m + 1], pb[:, 0:1])
                    pb2 = pffn.tile([128, 512], F32, tag="pf")
                    for c in range(NC3):
                        nc.tensor.matmul(
                            pb2[:, 0:2], wk[:, c, m * 128 : (m + 1) * 128],
                            be1c[:, c, :],
                            start=(c == 0), stop=(c == NC3 - 1),
                        )
                    nc.vector.tensor_copy(bkc[:, m : m + 1], pb2[:, 0:1])
                # bv as a broadcast tile: bv = be1 @ Wv'
                pbv = pffn.tile([128, 512], F32, tag="pf")
                for c in range(NC3):
                    nc.tensor.matmul(
                        pbv[0:1, 0:C], be1c[:, c, 0:1], wv[:, c, :],
                        start=(c == 0), stop=(c == NC3 - 1),
                    )
                nc.vector.tensor_copy(bv_row, pbv[0:1, 0:C])
                nc.gpsimd.partition_broadcast(bvb, bv_row)

            def emit_prep_p():
                nc.scalar.copy(
                    wp.rearrange("p a c -> p (a c)"),
                    wp_raw.rearrange("p a c -> p (a c)"),
                )

            def prep_ffn_chunks():
                chunks = []
                for a in range(NC3):
                    def gof(a=a):
                        nc.vector.tensor_scalar(
                            w1[:, a, :], w1_raws[a], g2c[:, a : a + 1], None, ALU.mult
                        )
                    chunks.append(gof)
                for half in range(2):
                    def goc(half=half):
                        nc.scalar.copy(
                            w2[:, 6 * half : 6 * half + 6, :].rearrange(
                                "p a c -> p (a c)"
                            ),
                            w2_raws[half].rearrange("p a c -> p (a c)"),
                        )
                    chunks.append(goc)
                # b1tot = b1 + W1'^T be2 (per-feature bias col for FFN1 evict)
                for m in range(NF):
                    def gob(m=m):
                        pb3 = pffn.tile([128, 512], F32, tag="pf")
                        for c in range(NC3):
                            nc.tensor.matmul(
                                pb3[:, 0:2], w1[:, c, m * 128 : (m + 1) * 128],
                                be2c[:, c, :],
                                start=(c == 0), stop=(c == NC3 - 1),
                            )
                        nc.vector.scalar_tensor_tensor(
                            b1tot[:, m : m + 1], pb3[:, 0:1], 1.0, b1c[:, m : m + 1],
                            ALU.mult, ALU.add,
                        )
                    chunks.append(gob)
                return chunks

            # ---------------- helpers ----------------
            def layernorm_tile(src_ap, dst_tile, apply_eng="act"):
                """LN stats on DVE, apply on ACT or DVE (dst = src*rstd + nmr)."""
                st6 = stats.tile([128, 6], F32, tag="st6")
                mv = stats.tile([128, 2], F32, tag="mv")
                nc.vector.bn_stats(st6, src_ap)
                nc.vector.bn_aggr(mv, st6)
                rstd = stats.tile([128, 1], F32, tag="rstd")
                nmr = stats.tile([128, 1], F32, tag="nmr")
                nc.scalar.activation(rstd, mv[:, 1:2], AF.Sqrt, bias=eps_col, scale=1.0)
                nc.vector.reciprocal(rstd, rstd)
                nc.vector.scalar_tensor_tensor(
                    nmr, mv[:, 0:1], -1.0, rstd, ALU.mult, ALU.mult
                )
                if apply_eng == "act":
                    nc.scalar.activation(
                        dst_tile, src_ap, AF.Identity, bias=nmr, scale=rstd
                    )
                else:
                    nc.vector.tensor_scalar(
                        dst_tile, src_ap, rstd, nmr, ALU.mult, ALU.add
                    )

            def transpose3(src_tile, i, evict_engine):
                """Transpose [128, 384] natural tile into slab cols i*128..,
                via 3 PE transposes into one PSUM bank + one wide eviction."""
                pt = pbig.tile([128, NC3, 128], src_tile.dtype, tag="pb")
                for c in range(NC3):
                    nc.tensor.transpose(
                        pt[:, c, :], src_tile[:, c * 128 : (c + 1) * 128], ident
                    )
                dst = slab[:, :, i * 128 : (i + 1) * 128]
                if evict_engine == "act":
                    nc.scalar.copy(dst, pt)
                else:
                    nc.vector.tensor_copy(dst, pt)

            def emit_scores(b, hp, store):
                """pair scores + exp + mask for head pair hp of batch b."""
                col = b * T
                e01s = []
                pws = []
                for h in (2 * hp, 2 * hp + 1):
                    jj, r0 = h // 2, (h % 2) * 64
                    pw = pwp.tile([128, 384], F32, tag="pw")
                    nc.tensor.matmul(
                        pw[:, 0:128],
                        kT[r0 : r0 + 64, jj, col + 128 : col + 256],
                        qT[r0 : r0 + 64, jj, col + 128 : col + 256],
                        start=True, stop=True,
                    )
                    nc.tensor.matmul(
                        pw[:, 128:384],
                        kT[r0 : r0 + 64, jj, col : col + 128],
                        qT[r0 : r0 + 64, jj, col : col + 256],
                        start=True, stop=True,
                    )
                    pws.append(pw)
                for pw in pws:
                    e01 = expb.tile([128, 384], DT_E, tag="e01")
                    nc.scalar.activation(e01, pw, AF.Exp)
                    # both diagonal blocks are contiguous: one mask op
                    nc.vector.tensor_mul(e01[:, 0:256], e01[:, 0:256], mask2x)
                    e01s.append(e01)
                store[hp] = e01s

            def emit_av(b, hp, po0, po1, store):
                for idx, h in enumerate((2 * hp, 2 * hp + 1)):
                    e01 = store[hp][idx]
                    nc.tensor.matmul(
                        po0[:, h, :], e01[:, 128:256], v1[:, 2 * b, h, :],
                        start=True, stop=True,
                    )
                    nc.tensor.matmul(
                        po1[:, h, :], e01[:, 256:384], v1[:, 2 * b, h, :],
                        start=True, stop=False,
                    )
                    nc.tensor.matmul(
                        po1[:, h, :], e01[:, 0:128], v1[:, 2 * b + 1, h, :],
                        start=False, stop=True,
                    )

            def emit_norm(b, po0, po1):
                """normalize + evict + transpose oT into the slab."""
                for tch, po in enumerate((po0, po1)):
                    ost = ostp.tile([128, C], DT_SLAB, tag="ost")
                    rc = stats.tile([128, H], F32, tag="rc")
                    nc.vector.reciprocal(
                        rc, po[:, :, HS : HS + 1].rearrange("p h o -> p (h o)")
                    )
                    nc.vector.tensor_tensor(
                        ost.rearrange("p (h d) -> p h d", h=H),
                        po[:, :, 0:HS],
                        rc.unsqueeze(2).broadcast_to([128, H, HS]),
                        ALU.mult,
                    )
                    transpose3(ost, 2 * b + tch, "dve")

            def emit_wp(b, it):
                """Wp + residual + LN2 + h2T for one tile."""
                ps = pbig.tile([128, 512], F32, tag="pb")
                for c in range(NC3):
                    nc.tensor.matmul(
                        ps[:, 0:C],
                        slab[:, c, it * 128 : (it + 1) * 128],
                        wp[:, c, :],
                        start=(c == 0), stop=(c == NC3 - 1),
                    )
                # y = psum + (x + bp);  x += bp was added after LN1
                nc.vector.scalar_tensor_tensor(
                    xs[:, it, :], ps[:, 0:C], 1.0, xs[:, it, :],
                    ALU.mult, ALU.add,
                )
                h2_t = hb.tile([128, C], DT_SLAB, tag="h2")
                layernorm_tile(xs[:, it, :], h2_t, "dve" if it % 2 else "act")
                transpose3(h2_t, it, "act" if it % 2 else "dve")
                # y += b2 early (gpsimd latency hidden; FFN2 reads it late)
                nc.gpsimd.tensor_add(xs[:, it, :], xs[:, it, :], b2b)

            def attention_chunks(b):
                """attention for batch b as a list of emission closures."""
                po0 = povp.tile([128, H, VW], F32, tag="po")
                po1 = povp.tile([128, H, VW], F32, tag="po")
                store = {}
                chunks = []
                for hp in range(H // 2):
                    chunks.append(lambda hp=hp: emit_scores(b, hp, store))
                for hp in range(H // 2):
                    chunks.append(lambda hp=hp: emit_av(b, hp, po0, po1, store))
                chunks.append(lambda: emit_norm(b, po0, po1))
                chunks.append(lambda: emit_wp(b, 2 * b))
                chunks.append(lambda: emit_wp(b, 2 * b + 1))
                return chunks

            # ---------------- the pipeline ----------------
            def ln1_chunk(i):
                def go():
                    h_t = hb.tile([128, C], DT_SLAB, tag="h")
                    layernorm_tile(xs[:, i, :], h_t, "act" if i % 2 else "dve")
                    transpose3(h_t, i, "dve" if i % 2 else "act")
                    # x += bp early (gpsimd; Wp reads it much later)
                    nc.gpsimd.tensor_add(xs[:, i, :], xs[:, i, :], bpb)
                return go

            def qkv_chunks(j):
                """qkT m-block chunks + v tile chunks for token chunk j."""
                chunks = []
                for dst, w, bcol, eng in ((qT, wq, bqc, "dve"), (kT, wk, bkc, "act")):
                    for m in range(NC3):
                        def goqk(dst=dst, w=w, bcol=bcol, eng=eng, m=m):
                            pq = pbig.tile([128, 512], F32, tag="pb")
                            for c in range(NC3):
                                nc.tensor.matmul(
                                    pq,
                                    w[:, c, m * 128 : (m + 1) * 128],
                                    slab[:, c, j * 512 : (j + 1) * 512],
                                    start=(c == 0), stop=(c == NC3 - 1),
                                )
                            d = dst[:, m, j * 512 : (j + 1) * 512]
                            if eng == "dve":
                                nc.vector.tensor_scalar(
                                    d, pq, bcol[:, m : m + 1], None, ALU.add
                                )
                            else:
                                nc.scalar.activation(
                                    d, pq, AF.Identity,
                                    bias=bcol[:, m : m + 1], scale=1.0,
                                )
                        chunks.append(goqk)
                for it in range(4 * j, 4 * j + 4):
                    def gov(it=it):
                        pv = pbig.tile([128, 512], F32, tag="pb")
                        for c in range(NC3):
                            nc.tensor.matmul(
                                pv[:, 0:C],
                                slab[:, c, it * 128 : (it + 1) * 128],
                                wv[:, c, :],
                                start=(c == 0), stop=(c == NC3 - 1),
                            )
                        nc.vector.scalar_tensor_tensor(
                            v1[:, it, :, 0:HS],
                            pv[:, 0:C].rearrange("p (h d) -> p h d", h=H),
                            1.0,
                            bvb.rearrange("p (h d) -> p h d", h=H),
                            ALU.mult, ALU.add,
                        )
                    chunks.append(gov)
                return chunks

            def ffn1_chunk(j, hid, m):
                def go():
                    ph = pffn.tile([128, 512], F32, tag="pf")
                    for c in range(NC3):
                        nc.tensor.matmul(
                            ph,
                            w1[:, c, m * 128 : (m + 1) * 128],
                            slab[:, c, j * 512 : (j + 1) * 512],
                            start=(c == 0), stop=(c == NC3 - 1),
                        )
                    nc.vector.tensor_scalar(
                        hid[:, m, :], ph, b1tot[:, m : m + 1], 0.0,
                        ALU.add, ALU.max,
                    )
                return go

            def ffn2_chunk(j, hid, hold, sub, half):
                """half 0: first 6 accumulating matmuls; half 1: last 6 +
                eviction (+ DMA after odd tiles).  Splitting gives the weave
                finer granules; interleaved matmuls to other PSUM banks do
                not disturb this bank's has_written accumulation state."""
                def go():
                    it = 4 * j + sub
                    if half == 0:
                        if sub % 2 == 0:
                            hold["ob"] = outbp.tile(
                                [128, 2, C], F32, tag="ob", name="ob"
                            )
                        hold["pf"] = pffn.tile(
                            [128, 512], F32, tag="pf", name="pf"
                        )
                    pf = hold["pf"]
                    for m in range(6 * half, 6 * half + 6):
                        nc.tensor.matmul(
                            pf[:, 0:C],
                            hid[:, m, sub * 128 : (sub + 1) * 128],
                            w2[:, m, :],
                            start=(m == 0), stop=(m == NF - 1),
                        )
                    if half == 1:
                        ob = hold["ob"]
                        nc.vector.scalar_tensor_tensor(
                            ob[:, sub % 2, :], pf[:, 0:C], 1.0, xs[:, it, :],
                            ALU.mult, ALU.add,
                        )
                        if sub % 2 == 1:
                            nc.sync.dma_start(
                                out_v[:, it - 1 : it + 1, :], ob
                            )
                return go

            def ffn_chunks(j):
                hid = hidp.tile([128, NF, 512], DT_HID, tag="hid")
                hold = {}
                chunks = [ffn1_chunk(j, hid, m) for m in range(NF)]
                for sub in range(4):
                    chunks += [ffn2_chunk(j, hid, hold, sub, hf) for hf in range(2)]
                return chunks

            def weave(attn, fillers):
                """emit attention chunks with fillers spread between them."""
                nf, na = len(fillers), len(attn)
                fi = 0
                for ai, ch in enumerate(attn):
                    ch()
                    want = (ai + 1) * nf // na
                    while fi < want:
                        fillers[fi]()
                        fi += 1
                while fi < nf:
                    fillers[fi]()
                    fi += 1

            # PE warmup: ~5us of tiny matmuls at t=0 so the HAM clock-gate
            # reaches K=8/8 before the real work starts (it otherwise stays
            # at half clock through the whole prologue).
            def warmup():
                # tiny stationary (2 cols -> LDW ~free) + wide moving rhs:
                # ~90% array duty so the HAM activity monitor actually latches
                pwu = pffn.tile([128, 512], F32, tag="pf")
                for _ in range(40):
                    nc.tensor.matmul(
                        pwu[0:2, 0:128], ident[:, 0:2], ident,
                        start=True, stop=True,
                    )

            # prologue: LN1 of chunk 0 first (critical path), then weight
            # folds, then qkv(0)
            warmup()
            ln1_chunk(0)()
            ln1_chunk(1)()
            emit_prep_q()
            ln1_chunk(2)()
            emit_prep_k()
            ln1_chunk(3)()
            emit_prep_v()
            emit_prep_p()
            for ch in qkv_chunks(0):
                ch()
            # per-batch weave. LN1/qkv are front-loaded (they depend only on
            # x and the folded weights) so the early batches always have
            # dense PE filler work; each chunk-level FFN is split across the
            # two batches that follow it.
            fill_by_batch = [[] for _ in range(2 * NJ)]
            fill_by_batch[0] += prep_ffn_chunks()
            fill_by_batch[0] += [ln1_chunk(i) for i in range(4, 8)]
            fill_by_batch[0] += qkv_chunks(1)
            fill_by_batch[1] += [ln1_chunk(i) for i in range(8, 12)]
            fill_by_batch[1] += qkv_chunks(2)
            fill_by_batch[2] += [ln1_chunk(i) for i in range(12, 16)]
            fill_by_batch[3] += qkv_chunks(3)
            for b in range(2 * NJ):
                if b >= 2:
                    # ffn(j) is ready after batch 2j+1; split its 16 chunks
                    # over the next two batches
                    j = (b - 2) // 2
                    chunks = ffn_chunks(j) if b % 2 == 0 else None
                    if chunks is not None:
                        fill_by_batch[b] += chunks[:8]
                        fill_by_batch[b + 1] += chunks[8:]
                weave(attention_chunks(b), fill_by_batch[b])
            for ch in ffn_chunks(NJ - 1):
                ch()

    nc.finalize()
    return nc


_NC_CACHE = None


def _get_nc():
    global _NC_CACHE
    if _NC_CACHE is None:
        _NC_CACHE = build_nc()
    return _NC_CACHE


def make_in_maps(inputs):
    """Host-side input marshalling: pure reshapes/transposes, no math."""
    x = np.ascontiguousarray(np.asarray(inputs["x"], dtype=np.float32))
    wq = np.ascontiguousarray(
        np.asarray(inputs["Wq"], np.float32).transpose(1, 0, 2).reshape(C, C)
    )
    wk = np.ascontiguousarray(
        np.asarray(inputs["Wk"], np.float32).transpose(1, 0, 2).reshape(C, C)
    )
    wv = np.ascontiguousarray(
        np.asarray(inputs["Wv"], np.float32).transpose(1, 0, 2).reshape(C, C)
    )
    wp = np.ascontiguousarray(np.asarray(inputs["Wp"], np.float32))
    w1 = np.ascontiguousarray(np.asarray(inputs["W1"], np.float32))
    w2 = np.ascontiguousarray(np.asarray(inputs["W2"], np.float32))

    def col3(v):
        return np.ascontiguousarray(np.asarray(v, np.float32).reshape(NC3, 128).T)

    g1c = col3(inputs["g1"])
    be1c = col3(inputs["be1"])
    g2c = col3(inputs["g2"])
    be2c = col3(inputs["be2"])
    b1c = np.ascontiguousarray(np.asarray(inputs["b1"], np.float32).reshape(NF, 128).T)
    bp_row = np.asarray(inputs["bp"], np.float32).reshape(1, C)
    b2_row = np.asarray(inputs["b2"], np.float32).reshape(1, C)

    shared = dict(
        wq=wq, wk=wk, wv=wv, wp=wp, w1=w1, w2=w2,
        g1c=g1c, be1c=be1c, g2c=g2c, be2c=be2c, b1c=b1c,
        bp_row=bp_row, b2_row=b2_row,
    )
    in_maps = []
    for core in range(NCORES):
        m = dict(shared)
        m["x"] = np.ascontiguousarray(x[core * BB : (core + 1) * BB].reshape(N, C))
        in_maps.append(m)
    return in_maps


def kernel(**inputs):
    from concourse.bass_utils import run_bass_kernel_spmd

    nc = _get_nc()
    in_maps = make_in_maps(inputs)
    res = run_bass_kernel_spmd(nc, in_maps, list(range(NCORES)))
    outs = [
        np.asarray(res.results[i]["out"]).reshape(BB, T, C) for i in range(NCORES)
    ]
    return np.concatenate(outs, axis=0)


if __name__ == "__main__":
    nc = build_nc()
    print("built ok")
